# revision 27
# baseline (speedup 1.0000x reference)
"""CenterLoss Trainium2 kernel (8-core SPMD, data-parallel over batch).

loss = mean_i( ||feat_i - centers[label_i]|| / count[label_i] )

Device algorithm (per core, batch shard of 2048 rows, fp8 inputs):
  - host prep: shard by batch, sort each shard by label (gather reads HBM
    in ascending address order), downcast feat/centers to fp8 e4m3,
    negate feat (so a CCE-add DMA computes the subtraction).
  - per chunk: dma_gather center rows (fp8) -> G; SWDGE dma_start of the
    negated feat chunk with accum_op=add onto G  =>  G = c - f.
  - square+reduce per [128, D] tile, split ACT (Square + accum_out) /
    DVE (tensor_tensor_reduce mult+add) to balance engines.
  - dist = sqrt(dist2) on ACT (bf16 out).
  - radix-100 class factorization c = 100*h + l; one-hot A/B (bf16, DVE
    is_equal vs iota); PE matmuls: cnt2d[h,l] = sum_i A B and
    S2d[h,l] = sum_i A B dist_i.
  - host: sum partials over cores, loss = sum(S2d/max(cnt2d,1)) / B.
"""

from contextlib import ExitStack

import numpy as np
import ml_dtypes

import concourse.bass as bass
import concourse.tile as tile
from concourse import bacc, mybir
from concourse import bass_utils
from concourse.alu_op_type import AluOpType

B, D, C = 16384, 512, 10000
NCORES = 8
BLOC = B // NCORES  # 2048 rows per core
P = 128
TLOC = BLOC // P    # 16 local batch tiles
R = 100             # radix (c = 100*h + l)
DCHUNK = 4          # local tiles per dist DMA chunk
NDC = TLOC // DCHUNK

F32 = mybir.dt.float32
BF16 = mybir.dt.bfloat16
FP8 = mybir.dt.float8e4
I16 = mybir.dt.int16
NP_FP8 = ml_dtypes.float8_e4m3

# per-chunk split of the square+reduce work: first DVE_N[q] tiles of chunk q
# run on DVE (bn_stats), the rest on ACT (Square + accum_out)
DVE_N = (2, 2, 2, 3)
NP = 256          # consecutive-label sample pairs merged into 1024B gather descs
NB = BLOC - 2 * NP  # remaining singles (3 x 512-row gathers)

_CACHE: dict = {}


def build_program(reps: int = 1):
    """Build + compile the per-core Bass program (SPMD: same program on
    all 8 cores, different input data).

    reps > 1 repeats the whole body, chained through a scalar so DCE keeps
    every rep (for timing: marginal wall-clock per rep = pure device time).
    """
    nc = bacc.Bacc(
        "TRN2", target_bir_lowering=False, debug=False, enable_asserts=False
    )

    feat_d = nc.dram_tensor("featneg", [BLOC, D], FP8, kind="ExternalInput").ap()
    cent_d = nc.dram_tensor("centers", [C, D], FP8, kind="ExternalInput").ap()
    # meta packs [gidx (BLOC//16 cols) | hloc (TLOC) | lloc (TLOC)]: one
    # 128-descriptor DMA per rep instead of three
    MCOLS = NP // 16 + NB // 16 + 2 * TLOC
    meta_d = nc.dram_tensor("meta", [P, MCOLS], I16, kind="ExternalInput").ap()
    tok_d = nc.dram_tensor("tok", [1, 1], F32, kind="ExternalInput").ap()
    s_out_d = nc.dram_tensor("s_out", [R, R], F32, kind="ExternalOutput").ap()
    c_out_d = nc.dram_tensor("c_out", [R, R], F32, kind="ExternalOutput").ap()

    feat_r = feat_d.rearrange("(p t) d -> p t d", p=P)

    with tile.TileContext(nc) as tc, ExitStack() as ctx:
        const = ctx.enter_context(tc.tile_pool(name="const", bufs=2))
        big = ctx.enter_context(tc.tile_pool(name="big", bufs=8))
        work = ctx.enter_context(tc.tile_pool(name="work", bufs=8))
        fin = ctx.enter_context(tc.tile_pool(name="fin", bufs=2))
        psum = ctx.enter_context(tc.tile_pool(name="psum", bufs=3, space="PSUM"))

        # one-time constant: iota[p, h, j] = h (int16)
        iota_s = const.tile([P, R, TLOC], I16, tag="iota")
        nc.gpsimd.iota(
            iota_s[:], pattern=[[1, R], [0, TLOC]], base=0, channel_multiplier=0
        )

        chain_prev = None
        for _rep in range(reps):
            # ---- small input loads (one packed DMA)
            meta_s = const.tile([P, MCOLS], I16, tag="meta")
            nc.sync.dma_start(meta_s[:], meta_d[:])
            gidxa_s = meta_s[:, 0 : NP // 16]
            gidxb_s = meta_s[:, NP // 16 : NP // 16 + NB // 16]
            hloc_s = meta_s[:, NP // 16 + NB // 16 : NP // 16 + NB // 16 + TLOC]
            lloc_s = meta_s[:, NP // 16 + NB // 16 + TLOC : MCOLS]
            tok_s = const.tile([1, 1], F32, tag="tok")
            nc.sync.dma_start(tok_s[:], tok_d[:])

            # ---- local one-hots (bf16): no dist dependency, start early
            hloc_b = hloc_s.unsqueeze(1).broadcast_to([P, R, TLOC])
            lloc_b = lloc_s.unsqueeze(1).broadcast_to([P, R, TLOC])
            a_loc = fin.tile([P, R, TLOC], BF16, tag="a_loc")
            nc.vector.tensor_tensor(a_loc[:], hloc_b, iota_s[:], AluOpType.is_equal)
            b_loc = fin.tile([P, R, TLOC], BF16, tag="b_loc")
            nc.vector.tensor_tensor(b_loc[:], lloc_b, iota_s[:], AluOpType.is_equal)

            psum_cnt = psum.tile([R, R], F32, tag="psum_cnt")
            for t in range(TLOC):
                nc.tensor.matmul(
                    psum_cnt[:],
                    a_loc[:, :, t],
                    b_loc[:, :, t],
                    start=(t == 0),
                    stop=(t == TLOC - 1),
                )

            # ---- dist path fully pipelined per chunk:
            # gather (fp8 center rows, sorted idx) -> CCE-add feat chunk
            # (G = c - f) -> square+reduce (ACT/DVE split) -> sqrt ->
            # dist-scaled one-hots -> S matmuls
            psum_s = psum.tile([R, R], F32, tag="psum_s")
            PAIR = 2 * DCHUNK           # sqrt/scale/matmul granularity
            dist2_g = None
            for q in range(NDC):
                if q == 0:
                    # chunk 0 = pair-merged region: one 1024B-elem gather
                    # (2 consecutive center rows per descriptor)
                    ga = big.tile([P, NP // P, 2 * D], FP8, tag="ga")
                    # overlapping-window view of centers: row stride D,
                    # window 2*D (fetches rows base, base+1 per descriptor)
                    cent_pair = bass.AP(
                        cent_d.tensor, 0, [[D, C - 1], [1, 2 * D]]
                    )
                    nc.gpsimd.dma_gather(
                        out_ap=ga[:],
                        in_ap=cent_pair,
                        idxs_ap=gidxa_s,
                        num_idxs=NP,
                        num_idxs_reg=NP,
                        elem_size=2 * D,
                        elem_step=D,
                        single_packet=False,
                    )
                    nc.gpsimd.dma_start(
                        ga[:], feat_r[:, 0:DCHUNK], accum_op=AluOpType.add
                    )
                    cols = [
                        ga[:, t // 2, (t % 2) * D : (t % 2 + 1) * D]
                        for t in range(DCHUNK)
                    ]
                else:
                    gath_c = big.tile([P, DCHUNK, D], FP8, tag="gath")
                    nc.gpsimd.dma_gather(
                        out_ap=gath_c[:],
                        in_ap=cent_d[:],
                        idxs_ap=gidxb_s[:, (q - 1) * 32 : q * 32],
                        num_idxs=NB // 3,
                        num_idxs_reg=NB // 3,
                        elem_size=D,
                        single_packet=False,
                    )
                    # G += (-f)  => G = c - f   (CCE add during DMA)
                    nc.gpsimd.dma_start(
                        gath_c[:],
                        feat_r[:, q * DCHUNK : (q + 1) * DCHUNK],
                        accum_op=AluOpType.add,
                    )
                    cols = [gath_c[:, t] for t in range(DCHUNK)]
                if q % 2 == 0:
                    dist2_g = work.tile([P, PAIR], F32, tag="d2g")
                off = (q % 2) * DCHUNK
                nv = DVE_N[q]
                if nv:
                    # DVE path: bn_stats per tile -> [P, 6] (even/odd
                    # count, mean, count*var); dist2 = sum(M2) + 256*sum(mean^2)
                    stats = work.tile([P, DCHUNK, 6], F32, tag="stats")
                    for t in range(nv):
                        nc.vector.bn_stats(stats[:, t], cols[t])
                    means = stats[:, 0:nv, 1:6:3]
                    m2s = stats[:, 0:nv, 2:6:3]
                    msq = work.tile([P, DCHUNK, 2], F32, tag="msq")
                    nc.vector.tensor_tensor(
                        msq[:, 0:nv], means, means, AluOpType.mult
                    )
                    comb = work.tile([P, DCHUNK, 2], F32, tag="comb")
                    nc.vector.scalar_tensor_tensor(
                        out=comb[:, 0:nv],
                        in0=msq[:, 0:nv],
                        scalar=float(D // 2),
                        in1=m2s,
                        op0=AluOpType.mult,
                        op1=AluOpType.add,
                    )
                    nc.vector.tensor_reduce(
                        dist2_g[:, off : off + nv],
                        comb[:, 0:nv],
                        mybir.AxisListType.X,
                        AluOpType.add,
                    )
                for t in range(nv, DCHUNK):
                    sq = work.tile([P, D], F32, tag="sq")
                    nc.scalar.activation(
                        sq[:],
                        cols[t],
                        mybir.ActivationFunctionType.Square,
                        accum_out=dist2_g[:, off + t : off + t + 1],
                    )
                if q % 2 == 1:
                    g0 = (q - 1) * DCHUNK  # first global tile of the pair
                    dist_bfc = work.tile([P, PAIR], BF16, tag="dbfc")
                    nc.scalar.activation(
                        dist_bfc[:], dist2_g[:], mybir.ActivationFunctionType.Sqrt
                    )
                    bp_c = work.tile([P, R, PAIR], BF16, tag="bpc")
                    nc.vector.tensor_tensor(
                        bp_c[:],
                        b_loc[:, :, g0 : g0 + PAIR],
                        dist_bfc[:].unsqueeze(1).broadcast_to([P, R, PAIR]),
                        AluOpType.mult,
                    )
                    for t in range(PAIR):
                        nc.tensor.matmul(
                            psum_s[:],
                            a_loc[:, :, g0 + t],
                            bp_c[:, :, t],
                            start=(g0 + t == 0),
                            stop=(g0 + t == TLOC - 1),
                        )
            cnt_sb = fin.tile([R, R], F32, tag="cnt_sb")
            nc.vector.tensor_copy(cnt_sb[:], psum_cnt[:])
            s_sb = fin.tile([R, R], F32, tag="s_sb")
            nc.vector.tensor_copy(s_sb[:], psum_s[:])
            # tok/prev chain keeps every rep live under DCE when reps > 1
            # (depends on both result matrices); per-rep work still pipelines
            prev = tok_s if _rep == 0 else chain_prev
            ch1 = fin.tile([1, 1], F32, tag=f"ch1_{_rep}")
            nc.vector.scalar_tensor_tensor(
                out=ch1[:],
                in0=prev[:],
                scalar=0.0,
                in1=s_sb[0:1, 0:1],
                op0=AluOpType.mult,
                op1=AluOpType.add,
            )
            ch2 = fin.tile([1, 1], F32, tag=f"ch2_{_rep}")
            nc.vector.scalar_tensor_tensor(
                out=ch2[:],
                in0=ch1[:],
                scalar=0.0,
                in1=cnt_sb[0:1, 0:1],
                op0=AluOpType.mult,
                op1=AluOpType.add,
            )
            chain_prev = ch2
        # write outputs once (last rep's values + chain dependency)
        nc.sync.dma_start(s_out_d[:], s_sb[:])
        nc.sync.dma_start(c_out_d[:], cnt_sb[:])
        # fold the chain into c_out so every rep stays live
        extra = fin.tile([1, 1], F32, tag="extra")
        nc.vector.scalar_tensor_tensor(
            out=extra[:],
            in0=chain_prev[:],
            scalar=0.0,
            in1=cnt_sb[0:1, 0:1],
            op0=AluOpType.mult,
            op1=AluOpType.add,
        )
        nc.sync.dma_start(c_out_d[0:1, 0:1], extra[:])

    nc.compile()
    return nc


def make_in_maps(feat, label, centers, tok=0.0):
    """Shard + lay out full inputs into the 8 per-core input maps.

    Per core: sort the shard by label (ascending-address gather), place
    sample g = t*128 + p at SBUF (partition p, column t) -- the native
    dma_gather output layout -- and pre-permute feat rows in HBM to land
    the same way.  feat is negated so the CCE-add DMA subtracts.
    """
    feat = np.ascontiguousarray(np.asarray(feat, dtype=np.float32))
    label = np.asarray(label, dtype=np.int32)
    centers = np.ascontiguousarray(np.asarray(centers, dtype=np.float32))

    cent8 = np.ascontiguousarray(centers.astype(NP_FP8))
    tok_arr = np.full((1, 1), tok, dtype=np.float32)

    def wrap16(v):
        w = np.ascontiguousarray(v.astype(np.int16).reshape(-1, 16).T)
        return np.ascontiguousarray(np.tile(w, (P // 16, 1)))

    def band_perm(n):
        """gather-position g -> sorted-local element, banded so each SDMA
        engine's descriptor stream walks one dense ascending label range
        (engine of partition p ~ 2*((p%32)//4) + p//64)."""
        eng = np.array([2 * ((p % 32) // 4) + (p // 64) for p in range(P)])
        parts = [np.where(eng == k)[0] for k in range(16)]
        pidx = np.empty(P, np.int64)
        for k in range(16):
            pidx[parts[k]] = np.arange(len(parts[k]))
        per_part = n // P
        per_eng = n // 16
        g = np.arange(n)
        p = g % P
        c = g // P
        return eng[p] * per_eng + pidx[p] * per_part + c

    in_maps = []
    for k in range(NCORES):
        lab_k = label[k * BLOC : (k + 1) * BLOC]
        order = np.argsort(lab_k, kind="stable")
        ls = lab_k[order]                        # sorted labels

        # greedy pairing: samples with labels (u, u+1) share one 1024B desc
        vals, starts, cnts = np.unique(ls, return_index=True, return_counts=True)
        idx_of = {int(u): i for i, u in enumerate(vals)}
        rem = cnts.copy()
        pairs = []
        for i, u in enumerate(vals):
            j = idx_of.get(int(u) + 1)
            if j is None:
                continue
            while rem[i] > 0 and rem[j] > 0 and len(pairs) < NP:
                pairs.append(
                    (starts[i] + cnts[i] - rem[i], starts[j] + cnts[j] - rem[j])
                )
                rem[i] -= 1
                rem[j] -= 1
        assert len(pairs) == NP, f"core {k}: only {len(pairs)} pairs"
        pa = np.array([a for a, _ in pairs])
        pb = np.array([b for _, b in pairs])
        used = np.zeros(BLOC, bool)
        used[pa] = True
        used[pb] = True
        restpos = np.where(~used)[0]             # ascending labels

        # slot map S[p, t] = sorted-position of the sample at tile (p, t);
        # engine-banded permutations within each gather block
        S = np.empty((P, TLOC), np.int64)
        g = np.arange(NP)
        prm = band_perm(NP)
        S[g % P, 2 * (g // P)] = pa[prm]
        S[g % P, 2 * (g // P) + 1] = pb[prm]
        pa = pa[prm]                             # pair bases in gather order
        prm512 = band_perm(512)
        restpos_banded = np.empty(NB, np.int64)
        for bblk in range(NB // 512):
            blk = restpos[bblk * 512 : (bblk + 1) * 512]
            j = np.arange(512)
            S[j % P, DCHUNK + bblk * DCHUNK + j // P] = blk[prm512]
            restpos_banded[bblk * 512 : (bblk + 1) * 512] = blk[prm512]
        restpos = restpos_banded                 # singles in gather order

        slab = ls[S]                             # label per slot [P, TLOC]
        fneg = np.ascontiguousarray(
            (-feat[k * BLOC : (k + 1) * BLOC][order[S.reshape(-1)]]).astype(NP_FP8)
        )
        meta = np.concatenate(
            [
                wrap16(ls[pa]),                  # pair base labels  [P, NP//16]
                wrap16(ls[restpos]),             # single labels     [P, NB//16]
                (slab // R).astype(np.int16),
                (slab % R).astype(np.int16),
            ],
            axis=1,
        )
        in_maps.append(
            {
                "featneg": fneg,
                "centers": cent8,
                "meta": np.ascontiguousarray(meta),
                "tok": tok_arr,
            }
        )
    return in_maps


def get_program():
    if "nc" not in _CACHE:
        _CACHE["nc"] = build_program()
    return _CACHE["nc"]


def kernel(feat, label, centers):
    nc = get_program()
    in_maps = make_in_maps(feat, label, centers)
    res = bass_utils.run_bass_kernel_spmd(nc, in_maps, core_ids=list(range(NCORES)))
    s_tot = np.zeros((R, R), dtype=np.float64)
    c_tot = np.zeros((R, R), dtype=np.float64)
    for k in range(NCORES):
        s_tot += res.results[k]["s_out"].astype(np.float64)
        c_tot += res.results[k]["c_out"].astype(np.float64)
    loss = (s_tot / np.maximum(c_tot, 1.0)).sum() / B
    return np.asarray(loss, dtype=np.float32)


# revision 28
# speedup vs baseline: 1.0374x; 1.0374x over previous
"""CenterLoss Trainium2 kernel (8-core SPMD, data-parallel over batch).

loss = mean_i( ||feat_i - centers[label_i]|| / count[label_i] )

Device algorithm (per core, batch shard of 2048 rows, fp8 inputs):
  - host prep: shard by batch, sort each shard by label (gather reads HBM
    in ascending address order), downcast feat/centers to fp8 e4m3,
    negate feat (so a CCE-add DMA computes the subtraction).
  - per chunk: dma_gather center rows (fp8) -> G; SWDGE dma_start of the
    negated feat chunk with accum_op=add onto G  =>  G = c - f.
  - square+reduce per [128, D] tile, split ACT (Square + accum_out) /
    DVE (tensor_tensor_reduce mult+add) to balance engines.
  - dist = sqrt(dist2) on ACT (bf16 out).
  - radix-100 class factorization c = 100*h + l; one-hot A/B (bf16, DVE
    is_equal vs iota); PE matmuls: cnt2d[h,l] = sum_i A B and
    S2d[h,l] = sum_i A B dist_i.
  - host: sum partials over cores, loss = sum(S2d/max(cnt2d,1)) / B.
"""

from contextlib import ExitStack

import numpy as np
import ml_dtypes

import concourse.bass as bass
import concourse.tile as tile
from concourse import bacc, mybir
from concourse import bass_utils
from concourse.alu_op_type import AluOpType

B, D, C = 16384, 512, 10000
NCORES = 8
BLOC = B // NCORES  # 2048 rows per core
P = 128
TLOC = BLOC // P    # 16 local batch tiles
R = 100             # radix (c = 100*h + l)
DCHUNK = 4          # local tiles per dist DMA chunk
NDC = TLOC // DCHUNK

F32 = mybir.dt.float32
BF16 = mybir.dt.bfloat16
FP8 = mybir.dt.float8e4
I16 = mybir.dt.int16
NP_FP8 = ml_dtypes.float8_e4m3

# per-chunk split of the square+reduce work: first DVE_N[q] tiles of chunk q
# run on DVE (bn_stats), the rest on ACT (Square + accum_out)
DVE_N = (2, 2, 2, 3)
NP = 256          # consecutive-label sample pairs merged into 1024B gather descs
NB = BLOC - 2 * NP  # remaining singles (3 x 512-row gathers)

_CACHE: dict = {}


def build_program(reps: int = 1):
    """Build + compile the per-core Bass program (SPMD: same program on
    all 8 cores, different input data).

    reps > 1 repeats the whole body, chained through a scalar so DCE keeps
    every rep (for timing: marginal wall-clock per rep = pure device time).
    """
    nc = bacc.Bacc(
        "TRN2", target_bir_lowering=False, debug=False, enable_asserts=False
    )

    feat_d = nc.dram_tensor("featneg", [BLOC, D], FP8, kind="ExternalInput").ap()
    cent_d = nc.dram_tensor("centers", [C, D], FP8, kind="ExternalInput").ap()
    # meta packs [gidx (BLOC//16 cols) | hloc (TLOC) | lloc (TLOC)]: one
    # 128-descriptor DMA per rep instead of three
    MCOLS = NP // 16 + NB // 16 + 2 * TLOC
    meta_d = nc.dram_tensor("meta", [P, MCOLS], I16, kind="ExternalInput").ap()
    tok_d = nc.dram_tensor("tok", [1, 1], F32, kind="ExternalInput").ap()
    s_out_d = nc.dram_tensor("s_out", [R, R], F32, kind="ExternalOutput").ap()
    c_out_d = nc.dram_tensor("c_out", [R, R], F32, kind="ExternalOutput").ap()

    feat_r = feat_d.rearrange("(p t) d -> p t d", p=P)

    with tile.TileContext(nc) as tc, ExitStack() as ctx:
        const = ctx.enter_context(tc.tile_pool(name="const", bufs=2))
        big = ctx.enter_context(tc.tile_pool(name="big", bufs=8))
        work = ctx.enter_context(tc.tile_pool(name="work", bufs=8))
        fin = ctx.enter_context(tc.tile_pool(name="fin", bufs=2))
        psum = ctx.enter_context(tc.tile_pool(name="psum", bufs=3, space="PSUM"))

        # one-time constant: iota[p, h, j] = h (int16)
        iota_s = const.tile([P, R, TLOC], I16, tag="iota")
        nc.gpsimd.iota(
            iota_s[:], pattern=[[1, R], [0, TLOC]], base=0, channel_multiplier=0
        )

        chain_prev = None
        for _rep in range(reps):
            # ---- small input loads (one packed DMA)
            meta_s = const.tile([P, MCOLS], I16, tag="meta")
            nc.sync.dma_start(meta_s[:], meta_d[:])
            gidxa_s = meta_s[:, 0 : NP // 16]
            gidxb_s = meta_s[:, NP // 16 : NP // 16 + NB // 16]
            hloc_s = meta_s[:, NP // 16 + NB // 16 : NP // 16 + NB // 16 + TLOC]
            lloc_s = meta_s[:, NP // 16 + NB // 16 + TLOC : MCOLS]
            tok_s = const.tile([1, 1], F32, tag="tok")
            nc.sync.dma_start(tok_s[:], tok_d[:])

            # ---- local one-hots (bf16): no dist dependency, start early
            hloc_b = hloc_s.unsqueeze(1).broadcast_to([P, R, TLOC])
            lloc_b = lloc_s.unsqueeze(1).broadcast_to([P, R, TLOC])
            a_loc = fin.tile([P, R, TLOC], BF16, tag="a_loc")
            nc.vector.tensor_tensor(a_loc[:], hloc_b, iota_s[:], AluOpType.is_equal)
            b_loc = fin.tile([P, R, TLOC], BF16, tag="b_loc")
            nc.vector.tensor_tensor(b_loc[:], lloc_b, iota_s[:], AluOpType.is_equal)

            psum_cnt = psum.tile([R, R], F32, tag="psum_cnt")
            for t in range(TLOC):
                nc.tensor.matmul(
                    psum_cnt[:],
                    a_loc[:, :, t],
                    b_loc[:, :, t],
                    start=(t == 0),
                    stop=(t == TLOC - 1),
                )

            # ---- dist path fully pipelined per chunk:
            # gather (fp8 center rows, sorted idx) -> CCE-add feat chunk
            # (G = c - f) -> square+reduce (ACT/DVE split) -> sqrt ->
            # dist-scaled one-hots -> S matmuls
            psum_s = psum.tile([R, R], F32, tag="psum_s")
            PAIR = 2 * DCHUNK           # sqrt/scale/matmul granularity
            dist2_g = None
            for q in range(NDC):
                if q == 0:
                    # chunk 0 = pair-merged region: one 1024B-elem gather
                    # (2 consecutive center rows per descriptor)
                    ga = big.tile([P, NP // P, 2 * D], FP8, tag="ga")
                    # overlapping-window view of centers: row stride D,
                    # window 2*D (fetches rows base, base+1 per descriptor)
                    cent_pair = bass.AP(
                        cent_d.tensor, 0, [[D, C - 1], [1, 2 * D]]
                    )
                    nc.gpsimd.dma_gather(
                        out_ap=ga[:],
                        in_ap=cent_pair,
                        idxs_ap=gidxa_s,
                        num_idxs=NP,
                        num_idxs_reg=NP,
                        elem_size=2 * D,
                        elem_step=D,
                        single_packet=False,
                    )
                    nc.gpsimd.dma_start(
                        ga[:], feat_r[:, 0:DCHUNK], accum_op=AluOpType.add
                    )
                    cols = [
                        ga[:, t // 2, (t % 2) * D : (t % 2 + 1) * D]
                        for t in range(DCHUNK)
                    ]
                else:
                    gath_c = big.tile([P, DCHUNK, D], FP8, tag="gath")
                    nc.gpsimd.dma_gather(
                        out_ap=gath_c[:],
                        in_ap=cent_d[:],
                        idxs_ap=gidxb_s[:, (q - 1) * 32 : q * 32],
                        num_idxs=NB // 3,
                        num_idxs_reg=NB // 3,
                        elem_size=D,
                        single_packet=False,
                    )
                    # G += (-f)  => G = c - f   (CCE add during DMA)
                    nc.gpsimd.dma_start(
                        gath_c[:],
                        feat_r[:, q * DCHUNK : (q + 1) * DCHUNK],
                        accum_op=AluOpType.add,
                    )
                    cols = [gath_c[:, t] for t in range(DCHUNK)]
                if q % 2 == 0:
                    dist2_g = work.tile([P, PAIR], F32, tag="d2g")
                off = (q % 2) * DCHUNK
                nv = DVE_N[q]
                if nv:
                    # DVE path: bn_stats per tile -> [P, 6] (even/odd
                    # count, mean, count*var); dist2 = sum(M2) + 256*sum(mean^2)
                    stats = work.tile([P, DCHUNK, 6], F32, tag="stats")
                    for t in range(nv):
                        nc.vector.bn_stats(stats[:, t], cols[t])
                    means = stats[:, 0:nv, 1:6:3]
                    m2s = stats[:, 0:nv, 2:6:3]
                    msq = work.tile([P, DCHUNK, 2], F32, tag="msq")
                    nc.vector.tensor_tensor(
                        msq[:, 0:nv], means, means, AluOpType.mult
                    )
                    comb = work.tile([P, DCHUNK, 2], F32, tag="comb")
                    nc.vector.scalar_tensor_tensor(
                        out=comb[:, 0:nv],
                        in0=msq[:, 0:nv],
                        scalar=float(D // 2),
                        in1=m2s,
                        op0=AluOpType.mult,
                        op1=AluOpType.add,
                    )
                    nc.vector.tensor_reduce(
                        dist2_g[:, off : off + nv],
                        comb[:, 0:nv],
                        mybir.AxisListType.X,
                        AluOpType.add,
                    )
                for t in range(nv, DCHUNK):
                    sq = work.tile([P, D], F32, tag="sq")
                    nc.scalar.activation(
                        sq[:],
                        cols[t],
                        mybir.ActivationFunctionType.Square,
                        accum_out=dist2_g[:, off + t : off + t + 1],
                    )
                if q % 2 == 1:
                    g0 = (q - 1) * DCHUNK  # first global tile of the pair
                    dist_bfc = work.tile([P, PAIR], BF16, tag="dbfc")
                    nc.scalar.activation(
                        dist_bfc[:], dist2_g[:], mybir.ActivationFunctionType.Sqrt
                    )
                    bp_c = work.tile([P, R, PAIR], BF16, tag="bpc")
                    nc.vector.tensor_tensor(
                        bp_c[:],
                        b_loc[:, :, g0 : g0 + PAIR],
                        dist_bfc[:].unsqueeze(1).broadcast_to([P, R, PAIR]),
                        AluOpType.mult,
                    )
                    for t in range(PAIR):
                        nc.tensor.matmul(
                            psum_s[:],
                            a_loc[:, :, g0 + t],
                            bp_c[:, :, t],
                            start=(g0 + t == 0),
                            stop=(g0 + t == TLOC - 1),
                        )
            cnt_sb = fin.tile([R, R], F32, tag="cnt_sb")
            nc.vector.tensor_copy(cnt_sb[:], psum_cnt[:])
            s_sb = fin.tile([R, R], F32, tag="s_sb")
            nc.vector.tensor_copy(s_sb[:], psum_s[:])
            # tok/prev chain keeps every rep live under DCE when reps > 1
            # (depends on both result matrices); per-rep work still pipelines
            prev = tok_s if _rep == 0 else chain_prev
            ch1 = fin.tile([1, 1], F32, tag=f"ch1_{_rep}")
            nc.vector.scalar_tensor_tensor(
                out=ch1[:],
                in0=prev[:],
                scalar=0.0,
                in1=s_sb[0:1, 0:1],
                op0=AluOpType.mult,
                op1=AluOpType.add,
            )
            ch2 = fin.tile([1, 1], F32, tag=f"ch2_{_rep}")
            nc.vector.scalar_tensor_tensor(
                out=ch2[:],
                in0=ch1[:],
                scalar=0.0,
                in1=cnt_sb[0:1, 0:1],
                op0=AluOpType.mult,
                op1=AluOpType.add,
            )
            chain_prev = ch2
        # write outputs once (last rep's values + chain dependency)
        nc.sync.dma_start(s_out_d[:], s_sb[:])
        nc.sync.dma_start(c_out_d[:], cnt_sb[:])
        # fold the chain into c_out so every rep stays live
        extra = fin.tile([1, 1], F32, tag="extra")
        nc.vector.scalar_tensor_tensor(
            out=extra[:],
            in0=chain_prev[:],
            scalar=0.0,
            in1=cnt_sb[0:1, 0:1],
            op0=AluOpType.mult,
            op1=AluOpType.add,
        )
        nc.sync.dma_start(c_out_d[0:1, 0:1], extra[:])

    nc.compile()
    return nc


def make_in_maps(feat, label, centers, tok=0.0):
    """Shard + lay out full inputs into the 8 per-core input maps.

    Per core: sort the shard by label (ascending-address gather), place
    sample g = t*128 + p at SBUF (partition p, column t) -- the native
    dma_gather output layout -- and pre-permute feat rows in HBM to land
    the same way.  feat is negated so the CCE-add DMA subtracts.
    """
    feat = np.ascontiguousarray(np.asarray(feat, dtype=np.float32))
    label = np.asarray(label, dtype=np.int32)
    centers = np.ascontiguousarray(np.asarray(centers, dtype=np.float32))

    cent8 = np.ascontiguousarray(centers.astype(NP_FP8))
    tok_arr = np.full((1, 1), tok, dtype=np.float32)

    def wrap16(v):
        w = np.ascontiguousarray(v.astype(np.int16).reshape(-1, 16).T)
        return np.ascontiguousarray(np.tile(w, (P // 16, 1)))

    in_maps = []
    for k in range(NCORES):
        lab_k = label[k * BLOC : (k + 1) * BLOC]
        order = np.argsort(lab_k, kind="stable")
        ls = lab_k[order]                        # sorted labels

        # greedy pairing: samples with labels (u, u+1) share one 1024B desc
        vals, starts, cnts = np.unique(ls, return_index=True, return_counts=True)
        idx_of = {int(u): i for i, u in enumerate(vals)}
        rem = cnts.copy()
        pairs = []
        for i, u in enumerate(vals):
            j = idx_of.get(int(u) + 1)
            if j is None:
                continue
            while rem[i] > 0 and rem[j] > 0 and len(pairs) < NP:
                pairs.append(
                    (starts[i] + cnts[i] - rem[i], starts[j] + cnts[j] - rem[j])
                )
                rem[i] -= 1
                rem[j] -= 1
        assert len(pairs) == NP, f"core {k}: only {len(pairs)} pairs"
        pa = np.array([a for a, _ in pairs])
        pb = np.array([b for _, b in pairs])
        used = np.zeros(BLOC, bool)
        used[pa] = True
        used[pb] = True
        restpos = np.where(~used)[0]             # ascending labels

        # slot map S[p, t] = sorted-position of the sample at tile (p, t)
        S = np.empty((P, TLOC), np.int64)
        g = np.arange(NP)
        S[g % P, 2 * (g // P)] = pa
        S[g % P, 2 * (g // P) + 1] = pb
        gp = np.arange(NB)
        bblk = gp // 512
        j = gp % 512
        S[j % P, DCHUNK + bblk * DCHUNK + j // P] = restpos

        slab = ls[S]                             # label per slot [P, TLOC]
        fneg = np.ascontiguousarray(
            (-feat[k * BLOC : (k + 1) * BLOC][order[S.reshape(-1)]]).astype(NP_FP8)
        )
        meta = np.concatenate(
            [
                wrap16(ls[pa]),                  # pair base labels  [P, NP//16]
                wrap16(ls[restpos]),             # single labels     [P, NB//16]
                (slab // R).astype(np.int16),
                (slab % R).astype(np.int16),
            ],
            axis=1,
        )
        in_maps.append(
            {
                "featneg": fneg,
                "centers": cent8,
                "meta": np.ascontiguousarray(meta),
                "tok": tok_arr,
            }
        )
    return in_maps


def get_program():
    if "nc" not in _CACHE:
        _CACHE["nc"] = build_program()
    return _CACHE["nc"]


def kernel(feat, label, centers):
    nc = get_program()
    in_maps = make_in_maps(feat, label, centers)
    res = bass_utils.run_bass_kernel_spmd(nc, in_maps, core_ids=list(range(NCORES)))
    s_tot = np.zeros((R, R), dtype=np.float64)
    c_tot = np.zeros((R, R), dtype=np.float64)
    for k in range(NCORES):
        s_tot += res.results[k]["s_out"].astype(np.float64)
        c_tot += res.results[k]["c_out"].astype(np.float64)
    loss = (s_tot / np.maximum(c_tot, 1.0)).sum() / B
    return np.asarray(loss, dtype=np.float32)
